# revision 1
# baseline (speedup 1.0000x reference)
"""DETR3D cross-attention Bass kernel for 8 trn2 NeuronCores.

Strategy: queries sharded 8 ways ((batch, query-block) grid: 2 batches x 4
query slices of 225). Each core holds its batch's feature pyramid in HBM as
bf16 pixel-major tables (row=pixel, 256ch; one table per camera with its 3
levels concatenated, each level padded by 1 lead + 2 trail rows) and
sparse-gathers only the bilinear-corner pixel pairs it needs via the gpsimd
dma_gather custom op (int16 wrapped indices, 768 per instruction, one
instruction per camera x query-chunk). The (cam, level, corner) weighted
reduction runs on the tensor engine as diag(beta) matmuls accumulating in
fp32 PSUM, with 4 diagonals built per DVE op via broadcast APs. Projection,
attention weights, output projection and the positional-embedding MLP run
on-device in fp32. No collectives: each core produces a disjoint output
slice.
"""

import numpy as np
import ml_dtypes

import concourse.bacc as bacc
import concourse.bass as bass
import concourse.mybir as mybir
import concourse.tile as tile
from concourse.bass_utils import run_bass_kernel_spmd

F32 = mybir.dt.float32
BF16 = mybir.dt.bfloat16
I16 = mybir.dt.int16
I32 = mybir.dt.int32
ALU = mybir.AluOpType
ACTF = mybir.ActivationFunctionType

B, Q, C, N, L = 2, 900, 256, 6, 3
LV = [(116, 200), (58, 100), (29, 50)]            # (H, W) per level
ROWS_L = [h * w + 3 for h, w in LV]               # 1 lead + HW + 2 trail pad rows
LBASE = [0, ROWS_L[0], ROWS_L[0] + ROWS_L[1]]     # level base within a cam table
CAM_ROWS = sum(ROWS_L)                            # 30459 (< int16 max)
TOTAL_ROWS = N * CAM_ROWS
IMG_W, IMG_H = 1600.0, 928.0
EPS = 1e-5

NQ_CORE = Q // 4                                  # 225 queries per core
CHUNKS = [(0, 128), (128, 128)]                   # (row0, qc; chunk1 padded)
NQ_PAD = 256

# tuning knobs
N_QUEUES = 4
DMA_SCRATCH = 32768
GATHER_DT = BF16      # feature table dtype (F32 fallback for precision)

_CACHE = {}

# combined-tile constants: 18-col order is n*3+l (axis blocks of 18 for x|y)
def _crow():
    sc = [0.0] * 36; b1 = [0.0] * 36; b2 = [0.0] * 36; lo = [0.0] * 36
    w18 = [0.0] * 18; ka = [0.0] * 18; kb = [0.0] * 18
    for n in range(N):
        for l in range(L):
            c = n * 3 + l
            H_l, W_l = LV[l]
            sc[c] = W_l / IMG_W; sc[18 + c] = H_l / IMG_H
            b1[c] = W_l - 1.0;   b1[18 + c] = H_l - 1.0
            b2[c] = W_l - 2.0;   b2[18 + c] = H_l - 2.0
            lo[c] = -1.0;        lo[18 + c] = 0.0
            w18[c] = float(W_l)
            ka[c] = 1.0 + LBASE[l]
            kb[c] = 1.0 + LBASE[l] + W_l
    return np.array(sc + b1 + b2 + lo + w18 + ka + kb, np.float32)[None, :]

CROW = _crow()        # [1, 198]


def _build():
    nc = bacc.Bacc("TRN2", target_bir_lowering=False, debug=False,
                   num_swdge_queues=N_QUEUES,
                   dynamic_dma_scratch_size=DMA_SCRATCH)

    def dram(name, shape, dtype=F32, kind="ExternalInput"):
        return nc.dram_tensor(name, shape, dtype, kind=kind)

    featT = dram("featT", [TOTAL_ROWS, C], GATHER_DT)
    xqpk = dram("xqpk", [NQ_PAD, 2 * C])
    refp = dram("refp", [NQ_PAD, 4])
    l2iT = dram("l2iT", [4, 4 * N])
    Wapk = dram("Wapk", [128, 36]); ba = dram("ba", [1, 18])
    Wpk = dram("Wpk", [128, 4 * C])
    Wp1 = dram("Wp1", [3, C]); Bpk = dram("Bpk", [1, 3 * C])
    ident_in = dram("ident", [128, 128])
    identb_in = dram("identb", [128, 128], GATHER_DT)
    sel16_in = dram("sel16", [16, 128])
    crow_in = dram("crow", [1, CROW.shape[1]])
    onesr_in = dram("onesr", [1, 128])
    out_d = dram("out", [NQ_CORE, C], kind="ExternalOutput")

    v = nc.vector
    s = nc.scalar

    with tile.TileContext(nc) as tc:
        with (
            tc.tile_pool(name="cst", bufs=1) as cst,
            tc.tile_pool(name="wk", bufs=2) as wk,
            tc.tile_pool(name="gp", bufs=1) as gp,  # 12 distinct tags, all live
            tc.tile_pool(name="dg", bufs=12) as dg,
            tc.tile_pool(name="pT", bufs=4, space="PSUM") as pT,
            tc.tile_pool(name="pA", bufs=2, space="PSUM") as pA,
            tc.tile_pool(name="pO", bufs=1, space="PSUM") as pO,
            tc.tile_pool(name="pW", bufs=1, space="PSUM") as pW,
        ):
            # ---- constants ----
            # Critical path (index wrap) loads go FIRST on the Sync HWDGE
            # FIFO; heavy weight tensors go on the Scalar HWDGE FIFO so they
            # don't delay the gather critical path.
            ident = cst.tile([128, 128], F32, name="ident")
            nc.sync.dma_start(ident[:], ident_in.ap())
            ref4_c = []
            for ci, (row0, qc) in enumerate(CHUNKS):
                r4 = cst.tile([qc, 4], F32, name=f"ref4_{ci}")
                nc.sync.dma_start(r4[:], refp.ap()[row0:row0 + qc, :])
                ref4_c.append(r4)
            sel16 = cst.tile([16, 128], F32, name="sel16")
            nc.sync.dma_start(sel16[:], sel16_in.ap())
            l2i_t = cst.tile([4, 4 * N], F32, name="l2i_t")
            nc.sync.dma_start(l2i_t[:], l2iT.ap())
            ones1 = cst.tile([1, 128], F32, name="ones1")
            nc.sync.dma_start(ones1[:], onesr_in.ap())
            xqq_c = []
            for ci, (row0, qc) in enumerate(CHUNKS):
                xg = cst.tile([qc, 2 * C], F32, name=f"xqq_{ci}")
                nc.scalar.dma_start(xg[:], xqpk.ap()[row0:row0 + qc, :])
                xqq_c.append(xg)
            identb = cst.tile([128, 128], GATHER_DT, name="identb")
            nc.scalar.dma_start(identb[:], identb_in.ap())
            wapk_t = cst.tile([128, 36], F32, name="wapk_t")
            nc.scalar.dma_start(wapk_t[:], Wapk.ap())
            wa_t = [wapk_t[:, 0:18], wapk_t[:, 18:36]]
            ba_t = cst.tile([1, 18], F32, name="ba_t")
            nc.scalar.dma_start(ba_t[:], ba.ap())

            # broadcast the combined-constant row to all partitions (rank-1)
            crow_t = cst.tile([1, CROW.shape[1]], F32, name="crow_t")
            nc.sync.dma_start(crow_t[:], crow_in.ap())
            cb_ps = pT.tile([128, CROW.shape[1]], F32, name="cb_ps", tag="tp",
                            space="PSUM")
            nc.tensor.matmul(cb_ps[:], lhsT=ones1[:], rhs=crow_t[:],
                             start=True, stop=True)
            CB = cst.tile([128, CROW.shape[1]], F32, name="CB")
            s.copy(CB[:], cb_ps[:])
            SC36, B1, B2, LOA = CB[:, 0:36], CB[:, 36:72], CB[:, 72:108], CB[:, 108:144]
            W18y, KA18, KB18 = CB[:, 144:162], CB[:, 162:180], CB[:, 180:198]

            cd = []       # per-chunk tiles kept across phases
            g_tiles = {}  # (cam, chunk) -> gathered pixel-pair tile

            # ================= phase A: projection + indices =================
            for ci, (row0, qc) in enumerate(CHUNKS):
                d = {"qc": qc, "row0": row0}
                ref4 = ref4_c[ci]
                rT_ps = pT.tile([4, qc], F32, name="rT_ps", tag="tp", space="PSUM")
                nc.tensor.transpose(rT_ps[:], ref4[:], ident[:qc, :qc])
                refT4 = wk.tile([4, qc], F32, name="refT4")
                v.tensor_copy(refT4[:], rT_ps[:])
                d["refT"] = refT4[0:3, :]

                cam_ps = pT.tile([qc, 4 * N], F32, name="cam_ps", tag="tp", space="PSUM")
                nc.tensor.matmul(cam_ps[:], lhsT=refT4[:, :], rhs=l2i_t[:, :],
                                 start=True, stop=True)
                cam = wk.tile([qc, 4 * N], F32, name="cam")
                v.tensor_copy(cam[:], cam_ps[:])

                zc = wk.tile([qc, N], F32, name="zc")
                v.tensor_scalar(zc[:], cam[:, 2::4], EPS, None, op0=ALU.max)
                rz = wk.tile([qc, N], F32, name="rz")
                v.reciprocal(rz[:], zc[:])
                pxy = wk.tile([qc, 2 * N], F32, name="pxy")
                v.tensor_tensor(out=pxy[:, 0:N], in0=cam[:, 0::4], in1=rz[:], op=ALU.mult)
                v.tensor_tensor(out=pxy[:, N:2 * N], in0=cam[:, 1::4], in1=rz[:], op=ALU.mult)
                d["z"], d["pxy"] = cam[:, 2::4], pxy

                # broadcast pxy [qc, (axis,n)] over levels -> [qc, (axis,n,l)]
                pap = pxy[:, :]
                pxy_b = bass.AP(pap.tensor, pap.offset,
                                [pap.ap[0], [N, 2], [1, N], [0, L]])
                XY = wk.tile([qc, 36], F32, name="XY")
                v.tensor_tensor(out=XY[:], in0=pxy_b, in1=SC36[:qc, :], op=ALU.mult)
                v.tensor_scalar(XY[:], XY[:], -0.5, None, op0=ALU.add)

                xi32 = wk.tile([qc, 36], I32, name="xi32")
                v.tensor_copy(xi32[:], XY[:])
                FLT = wk.tile([qc, 36], F32, name="FLT")
                v.tensor_copy(FLT[:], xi32[:])
                DGT = wk.tile([qc, 36], F32, name="DGT")
                v.tensor_tensor(out=DGT[:], in0=FLT[:], in1=XY[:], op=ALU.is_gt)
                v.tensor_tensor(out=FLT[:], in0=FLT[:], in1=DGT[:], op=ALU.subtract)
                FRAC = wk.tile([qc, 36], F32, name="FRAC")
                v.tensor_tensor(out=FRAC[:], in0=XY[:], in1=FLT[:], op=ALU.subtract)
                W0 = wk.tile([qc, 36], F32, name="W0")
                s.activation(W0[:], FRAC[:], ACTF.Copy, bias=1.0, scale=-1.0)

                CA = wk.tile([qc, 36], F32, name="CA")   # xs | ysA
                v.tensor_tensor(out=CA[:], in0=FLT[:], in1=LOA[:qc, :], op=ALU.max)
                v.tensor_tensor(out=CA[:], in0=CA[:], in1=B1[:qc, :], op=ALU.min)
                YB = wk.tile([qc, 18], F32, name="YB")   # ysB
                v.tensor_scalar(YB[:], FLT[:, 18:36], -1.0, None, op0=ALU.max)
                v.tensor_tensor(out=YB[:], in0=YB[:], in1=B2[:qc, 18:36], op=ALU.min)

                # indices (cols n*6 + l*2 + y in idxf)
                idxf = wk.tile([128, 36], F32, name="idxf")
                xsK = wk.tile([qc, 18], F32, name="xsK")
                tmp18 = wk.tile([qc, 18], F32, name="tmp18")
                v.tensor_tensor(out=xsK[:], in0=CA[:, 0:18], in1=KA18[:qc, :], op=ALU.add)
                v.tensor_tensor(out=tmp18[:], in0=CA[:, 18:36], in1=W18y[:qc, :], op=ALU.mult)
                v.tensor_tensor(out=idxf[:qc, 0::2], in0=tmp18[:], in1=xsK[:], op=ALU.add)
                v.tensor_tensor(out=xsK[:], in0=CA[:, 0:18], in1=KB18[:qc, :], op=ALU.add)
                v.tensor_tensor(out=tmp18[:], in0=YB[:], in1=W18y[:qc, :], op=ALU.mult)
                v.tensor_tensor(out=idxf[:qc, 1::2], in0=tmp18[:], in1=xsK[:], op=ALU.add)

                # valids are off the index critical path; emit after idxf
                ta = wk.tile([qc, 36], F32, name="ta")
                tb = wk.tile([qc, 36], F32, name="tb")
                V0 = wk.tile([qc, 36], F32, name="V0")
                V1 = wk.tile([qc, 36], F32, name="V1")
                v.tensor_scalar(ta[:], FLT[:], 0.0, None, op0=ALU.is_ge)
                v.tensor_tensor(out=tb[:], in0=FLT[:], in1=B1[:qc, :], op=ALU.is_le)
                v.tensor_tensor(out=V0[:], in0=ta[:], in1=tb[:], op=ALU.mult)
                v.tensor_scalar(ta[:], FLT[:], -1.0, None, op0=ALU.is_ge)
                v.tensor_tensor(out=tb[:], in0=FLT[:], in1=B2[:qc, :], op=ALU.is_le)
                v.tensor_tensor(out=V1[:], in0=ta[:], in1=tb[:], op=ALU.mult)
                for nm in ("FRAC", "W0", "V0", "V1"):
                    d[nm] = locals()[nm]
                cd.append(d)

                # -- wrap chunk indices: wall[pl, n*48+(l*2+y)*8+ph] --
                t1_ps = pT.tile([36, 128], F32, name="t1_ps", tag="tp", space="PSUM")
                nc.tensor.transpose(t1_ps[:], idxf[:], ident[:, :])
                t1s = wk.tile([36, 128], F32, name="t1s")
                v.tensor_copy(t1s[:], t1_ps[:])
                wallf = wk.tile([16, N * 48], F32, name="wallf")
                for ph in range(8):
                    t3_ps = pT.tile([16, 36], F32, name="t3_ps", tag="tp", space="PSUM")
                    nc.tensor.transpose(t3_ps[:], t1s[:, ph * 16:(ph + 1) * 16],
                                        ident[:36, :36])
                    wap = wallf[:, :]
                    dst = bass.AP(wap.tensor, wap.offset + ph,
                                  [wap.ap[0], [48, N], [16, L], [8, 2]])
                    v.tensor_copy(dst, t3_ps[:])
                wall_ps = pW.tile([128, N * 48], F32, name="wall_ps", tag="wall",
                                  space="PSUM")
                nc.tensor.matmul(wall_ps[:], lhsT=sel16[:], rhs=wallf[:],
                                 start=True, stop=True)
                wall = wk.tile([128, N * 48], I16, name="wall")
                v.tensor_copy(wall[:], wall_ps[:])
                d["wall"] = wall

                # -- phase C: attention weights + beta for this chunk --
                xqq = xqq_c[ci]
                xc_t = wk.tile([qc, C], F32, name="xc_t")
                v.tensor_tensor(out=xc_t[:], in0=xqq[:, 0:C], in1=xqq[:, C:2 * C], op=ALU.add)
                xT = []
                for k in range(2):
                    xT_ps = pT.tile([128, qc], F32, name="xT_ps", tag="tp", space="PSUM")
                    nc.tensor.transpose(xT_ps[:], xc_t[:, k * 128:(k + 1) * 128],
                                        ident[:qc, :qc])
                    xTk = wk.tile([128, qc], F32, name=f"xT{k}")
                    s.copy(xTk[:], xT_ps[:])
                    xT.append(xTk)
                at_ps = pT.tile([qc, 18], F32, name="at_ps", tag="tp", space="PSUM")
                nc.tensor.matmul(at_ps[:], lhsT=xT[0][:], rhs=wa_t[0], start=True, stop=False)
                nc.tensor.matmul(at_ps[:], lhsT=xT[1][:], rhs=wa_t[1], start=False, stop=False)
                nc.tensor.matmul(at_ps[:], lhsT=ones1[:1, :qc], rhs=ba_t[:], start=False, stop=True)
                w_t = wk.tile([qc, 18], F32, name="w_t")
                s.activation(w_t[:], at_ps[:], ACTF.Sigmoid)

                mask = wk.tile([qc, N], F32, name="mask")
                tm = wk.tile([qc, N], F32, name="tm")
                v.tensor_scalar(mask[:], d["z"], EPS, None, op0=ALU.is_gt)
                pxy = d["pxy"]
                for (sl, op, thr) in ((slice(0, N), ALU.is_gt, 0.0),
                                      (slice(0, N), ALU.is_lt, IMG_W),
                                      (slice(N, 2 * N), ALU.is_gt, 0.0),
                                      (slice(N, 2 * N), ALU.is_lt, IMG_H)):
                    v.tensor_scalar(tm[:], pxy[:, sl], thr, None, op0=op)
                    v.tensor_tensor(out=mask[:], in0=mask[:], in1=tm[:], op=ALU.mult)

                # A = w * mask (mask broadcast over levels)
                A18 = wk.tile([qc, 18], F32, name="A18")
                map_ = mask[:, :]
                mask_b = bass.AP(map_.tensor, map_.offset, [map_.ap[0], [1, N], [0, L]])
                v.tensor_tensor(out=A18[:], in0=w_t[:], in1=mask_b, op=ALU.mult)

                FRAC, W0, V0, V1 = d["FRAC"], d["W0"], d["V0"], d["V1"]
                m0 = wk.tile([qc, 18], F32, name="m0")
                m1 = wk.tile([qc, 18], F32, name="m1")
                u0 = wk.tile([qc, 18], F32, name="u0")
                u1 = wk.tile([qc, 18], F32, name="u1")
                v.tensor_tensor(out=m0[:], in0=W0[:, 18:36], in1=V0[:, 18:36], op=ALU.mult)
                v.tensor_tensor(out=m0[:], in0=m0[:], in1=A18[:], op=ALU.mult)
                v.tensor_tensor(out=m1[:], in0=FRAC[:, 18:36], in1=V1[:, 18:36], op=ALU.mult)
                v.tensor_tensor(out=m1[:], in0=m1[:], in1=A18[:], op=ALU.mult)
                v.tensor_tensor(out=u0[:], in0=W0[:, 0:18], in1=V0[:, 0:18], op=ALU.mult)
                v.tensor_tensor(out=u1[:], in0=FRAC[:, 0:18], in1=V1[:, 0:18], op=ALU.mult)
                beta = wk.tile([qc, 72], F32, name="beta")
                v.tensor_tensor(out=beta[:, 0::4], in0=m0[:], in1=u0[:], op=ALU.mult)
                v.tensor_tensor(out=beta[:, 1::4], in0=m0[:], in1=u1[:], op=ALU.mult)
                v.tensor_tensor(out=beta[:, 2::4], in0=m1[:], in1=u0[:], op=ALU.mult)
                v.tensor_tensor(out=beta[:, 3::4], in0=m1[:], in1=u1[:], op=ALU.mult)
                if GATHER_DT == BF16:
                    beta_c = wk.tile([qc, 72], BF16, name="beta_c")
                    v.tensor_copy(beta_c[:], beta[:])
                else:
                    beta_c = beta
                d["beta"] = beta_c


            # ================= gathers: one per (cam, chunk) =================
            for n in range(N):
                for ci in range(2):
                    wall = cd[ci]["wall"]
                    src = bass.AP(featT.ap().tensor, n * CAM_ROWS * C,
                                  [[C, CAM_ROWS - 1], [1, 2 * C]])
                    g_t = gp.tile([128, 6 * 2 * C], GATHER_DT, name=f"g_{n}_{ci}",
                                  tag=f"g{n}_{ci}")
                    g3 = g_t[:].rearrange("p (i r) -> p i r", i=6)
                    nc.gpsimd.dma_gather(
                        out_ap=g3, in_ap=src,
                        idxs_ap=wall[:, n * 48:(n + 1) * 48],
                        num_idxs=768, num_idxs_reg=768,
                        elem_size=2 * C, elem_step=C,
                        queue_num=(n * 2 + ci) % N_QUEUES,
                    )
                    g_tiles[(n, ci)] = g_t

            # ---- deferred weight loads (used only in phase E; 3 dispatches) ----
            wpk_t = cst.tile([128, 4 * C], F32, name="wpk_t")
            nc.sync.dma_start(wpk_t[:], Wpk.ap())
            wo_t = [wpk_t[:, 0:C], wpk_t[:, C:2 * C]]
            wp2_t = [wpk_t[:, 2 * C:3 * C], wpk_t[:, 3 * C:4 * C]]
            wp1_t = cst.tile([3, C], F32, name="wp1_t")
            nc.sync.dma_start(wp1_t[:], Wp1.ap())
            bpk_t = cst.tile([1, 3 * C], F32, name="bpk_t")
            nc.sync.dma_start(bpk_t[:], Bpk.ap())
            bo_t = bpk_t[:, 0:C]
            bp1_t = bpk_t[:, C:2 * C]
            bp2_t = bpk_t[:, 2 * C:3 * C]

            # ====== phase D: weighted reduction (PE diag matmuls) ======
            accp = []
            for ci, (row0, qc) in enumerate(CHUNKS):
                accp.append(pA.tile([qc, C], F32, name="accp", tag="accp", space="PSUM"))
            idb = identb if GATHER_DT == BF16 else ident
            n_mm = N * L * 4
            seen = [0, 0]
            for n in range(N):
                for l in range(L):
                    for ci, (row0, qc) in enumerate(CHUNKS):
                        beta = cd[ci]["beta"]
                        t0 = n * 12 + l * 4
                        D4 = dg.tile([128, 4 * 128], GATHER_DT, name="D4", tag="d4")
                        iap = idb[:qc, :qc]
                        ident_b = bass.AP(iap.tensor, iap.offset,
                                          [iap.ap[0], [0, 4], [1, qc]])
                        bap = beta[:, t0:t0 + 4]
                        beta_b = bass.AP(bap.tensor, bap.offset,
                                         [bap.ap[0], [1, 4], [0, qc]])
                        v.tensor_tensor(out=D4[:qc, 0:4 * qc], in0=ident_b, in1=beta_b,
                                        op=ALU.mult)
                        for y in range(2):
                            for xc in range(2):
                                t = y * 2 + xc
                                gsl = g_tiles[(n, ci)][:qc,
                                        (l * 2 + y) * 512 + xc * 256:
                                        (l * 2 + y) * 512 + (xc + 1) * 256]
                                k = seen[ci]; seen[ci] += 1
                                nc.tensor.matmul(accp[ci][:],
                                                 lhsT=D4[:qc, t * qc:(t + 1) * qc],
                                                 rhs=gsl, start=(k == 0),
                                                 stop=(k == n_mm - 1))
            for ci, (row0, qc) in enumerate(CHUNKS):
                fused = wk.tile([qc, C], F32, name="fused")
                s.copy(fused[:], accp[ci][:])
                cd[ci]["fused"] = fused

            # ====== phase E: output projection + positional MLP ======
            for ci, (row0, qc) in enumerate(CHUNKS):
                d = cd[ci]
                out_ps = pO.tile([qc, C], F32, name="out_ps", tag="outp", space="PSUM")
                for k in range(2):
                    fT_ps = pT.tile([128, qc], F32, name="fT_ps", tag="tp", space="PSUM")
                    nc.tensor.transpose(fT_ps[:], d["fused"][:, k * 128:(k + 1) * 128],
                                        ident[:qc, :qc])
                    fTk = wk.tile([128, qc], F32, name=f"fT{k}")
                    s.copy(fTk[:], fT_ps[:])
                    nc.tensor.matmul(out_ps[:], lhsT=fTk[:], rhs=wo_t[k],
                                     start=(k == 0), stop=False)
                nc.tensor.matmul(out_ps[:], lhsT=ones1[:1, :qc], rhs=bo_t,
                                 start=False, stop=False)

                refT = d["refT"]
                c01 = wk.tile([3, qc], F32, name="c01")
                v.tensor_scalar(c01[:], refT[:], 0.0, 1.0, op0=ALU.max, op1=ALU.min)
                x1 = wk.tile([3, qc], F32, name="x1")
                v.tensor_scalar(x1[:], c01[:], EPS, None, op0=ALU.max)
                x2 = wk.tile([3, qc], F32, name="x2")
                s.activation(x2[:], c01[:], ACTF.Copy, bias=1.0, scale=-1.0)
                v.tensor_scalar(x2[:], x2[:], EPS, None, op0=ALU.max)
                v.reciprocal(x2[:], x2[:])
                v.tensor_tensor(out=x1[:], in0=x1[:], in1=x2[:], op=ALU.mult)
                isgT = wk.tile([3, qc], F32, name="isgT")
                s.activation(isgT[:], x1[:], ACTF.Ln)

                h_ps = pT.tile([qc, C], F32, name="h_ps", tag="tp", space="PSUM")
                nc.tensor.matmul(h_ps[:], lhsT=isgT[:], rhs=wp1_t[:], start=True, stop=False)
                nc.tensor.matmul(h_ps[:], lhsT=ones1[:1, :qc], rhs=bp1_t,
                                 start=False, stop=True)
                h_t = wk.tile([qc, C], F32, name="h_t")
                s.activation(h_t[:], h_ps[:], ACTF.Relu)
                for k in range(2):
                    hT_ps = pT.tile([128, qc], F32, name="hT_ps", tag="tp", space="PSUM")
                    nc.tensor.transpose(hT_ps[:], h_t[:, k * 128:(k + 1) * 128],
                                        ident[:qc, :qc])
                    hTk = wk.tile([128, qc], F32, name=f"hT{k}")
                    s.copy(hTk[:], hT_ps[:])
                    nc.tensor.matmul(out_ps[:], lhsT=hTk[:], rhs=wp2_t[k],
                                     start=False, stop=False)
                nc.tensor.matmul(out_ps[:], lhsT=ones1[:1, :qc], rhs=bp2_t,
                                 start=False, stop=True)

                out_sb = wk.tile([qc, C], F32, name="out_sb")
                s.copy(out_sb[:], out_ps[:])
                take = min(qc, NQ_CORE - row0)
                nc.sync.dma_start(out_d.ap()[row0:row0 + take, :], out_sb[:take, :])

    nc.compile()
    return nc


def _host_prep(inputs):
    feats = [inputs["feat0"], inputs["feat1"], inputs["feat2"]]
    lidar2img = np.asarray(inputs["lidar2img"], np.float32)
    tdt = ml_dtypes.bfloat16 if GATHER_DT == BF16 else np.float32

    featT_b = []
    for b in range(B):
        tbl = np.zeros((TOTAL_ROWS, C), tdt)
        for n in range(N):
            for l in range(L):
                h, w = LV[l]
                base = n * CAM_ROWS + LBASE[l] + 1
                f = np.asarray(feats[l][b, n], np.float32)       # [C, H, W]
                tbl[base:base + h * w] = f.reshape(C, h * w).T.astype(tdt)
        featT_b.append(tbl)

    # fold reference-point denormalization into the projection matrices:
    # cam = lidar2img @ D @ [r; 1], D = affine denorm to pc_range
    Dmat = np.array([[102.4, 0, 0, -51.2],
                     [0, 102.4, 0, -51.2],
                     [0, 0, 8.0, -5.0],
                     [0, 0, 0, 1.0]], np.float32)
    l2iT_b = []
    for b in range(B):
        M = np.einsum('nij,jk->nik', lidar2img[b].astype(np.float64),
                      Dmat.astype(np.float64)).astype(np.float32)  # [N,4,4]
        l2iT_b.append(np.ascontiguousarray(M.transpose(2, 0, 1).reshape(4, 4 * N)))
    ident = np.eye(128, dtype=np.float32)
    in_maps = []
    for core in range(8):
        b, sidx = core // 4, core % 4
        rows = slice(sidx * NQ_CORE, (sidx + 1) * NQ_CORE)
        in_maps.append({
            "featT": featT_b[b],
            "xqpk": np.concatenate([
                np.pad(np.asarray(inputs["query"][b, rows], np.float32),
                       ((0, NQ_PAD - NQ_CORE), (0, 0))),
                np.pad(np.asarray(inputs["query_pos"][b, rows], np.float32),
                       ((0, NQ_PAD - NQ_CORE), (0, 0)))], axis=1),
            "refp": np.pad(np.concatenate(
                [np.asarray(inputs["reference_points"][b, rows], np.float32),
                 np.ones((NQ_CORE, 1), np.float32)], axis=1),
                ((0, NQ_PAD - NQ_CORE), (0, 0)), constant_values=0.5),
            "l2iT": l2iT_b[b],
            "Wapk": np.concatenate([np.asarray(inputs["W_attn"], np.float32)[0:128],
                                    np.asarray(inputs["W_attn"], np.float32)[128:256]], axis=1),
            "ba": np.asarray(inputs["b_attn"], np.float32).reshape(1, 18),
            "Wpk": np.concatenate([np.asarray(inputs["W_out"], np.float32)[0:128],
                                   np.asarray(inputs["W_out"], np.float32)[128:256],
                                   np.asarray(inputs["W_pe2"], np.float32)[0:128],
                                   np.asarray(inputs["W_pe2"], np.float32)[128:256]], axis=1),
            "Wp1": np.asarray(inputs["W_pe1"], np.float32),
            "Bpk": np.concatenate([np.asarray(inputs["b_out"], np.float32),
                                   np.asarray(inputs["b_pe1"], np.float32),
                                   np.asarray(inputs["b_pe2"], np.float32)])[None, :].astype(np.float32),
            "ident": ident,
            "identb": ident.astype(tdt),
            "sel16": (np.arange(128)[None, :] % 16 == np.arange(16)[:, None]).astype(np.float32),
            "crow": CROW,
            "onesr": np.ones((1, 128), np.float32),
        })
    return in_maps


def kernel(**inputs) -> np.ndarray:
    if "nc" not in _CACHE:
        _CACHE["nc"] = _build()
    nc = _CACHE["nc"]
    in_maps = _host_prep(inputs)
    res = run_bass_kernel_spmd(nc, in_maps, core_ids=list(range(8)),
                               **_CACHE.get("run_kwargs", {}))
    _CACHE["last_results"] = res
    out = np.zeros((B, Q, C), np.float32)
    for core in range(8):
        b, sidx = core // 4, core % 4
        out[b, sidx * NQ_CORE:(sidx + 1) * NQ_CORE] = res.results[core]["out"]
    return out

